# revision 2
# baseline (speedup 1.0000x reference)
"""Expert-parallel MoE feed-forward (top-2 routing) on 8 TRN2 NeuronCores.

Strategy: one expert per core (E == n_cores == 8). Token routing is part of
input sharding: host gathers each expert's assigned token activations
(transposed, bf16) and feeds core e only its tokens plus its expert's three
weight matrices. Each core runs a dense FFN
    out = (silu(x @ Wg^T) * (x @ Wu^T)) @ Wd^T
over its token batch in bf16 (fp32 PSUM accumulation), entirely from SBUF.
Host scatters per-core outputs back into the (T, A, D) result.

Input DMAs are batched into few multi-k-tile transfers (each dma_start costs
~0.6 us of serial sequencer issue time regardless of size) ordered by the
PE's consumption deadline, spread over the three DMA-capable engines
(SP/ACT hardware DGE + GpSimd software DGE) so arrival keeps ahead of the
gate phase's k-major weight consumption during the startup ramp.
"""

import math
import sys
import types

import numpy as np
import ml_dtypes

T, D, H, E, A = 4096, 1024, 2048, 8, 2
N_CORES = 8
BF16 = ml_dtypes.bfloat16

# Filled by kernel() with the BassKernelResults of the last device run so an
# external harness (test.py) can read exec_time_ns when tracing is on.
LAST_RESULT = None

_SHIMS_DONE = False


def _install_shims():
    """Environment fixes for running Bass/Tile SPMD kernels under axon."""
    global _SHIMS_DONE
    if _SHIMS_DONE:
        return
    _SHIMS_DONE = True

    # 1. NTFF profile hook (lets trace=True / BASS_TRACE=1 report exec_time_ns).
    if "antenv.axon_hooks" not in sys.modules:
        try:
            import antenv.axon_hooks  # noqa: F401  (real module present)
        except ImportError:
            _hook = None
            try:
                import trn_agent_boot.trn_boot as tb

                _hook = tb._ntff_profile_via_ctypes("/opt/axon/libaxon_pjrt.so")
            except Exception:
                _hook = None
            mod = types.ModuleType("antenv.axon_hooks")
            mod.get_axon_ntff_profile_hook = lambda: _hook
            sys.modules["antenv.axon_hooks"] = mod

    # 2. No artifact upload from a zero-egress container.
    from concourse import bass_utils

    bass_utils.upload_artifacts = lambda tmpdir: f"local:{tmpdir}"

    # 3. Slim tile-exit: keep the drain (its sem waits are what guarantee the
    # final output DMAs have landed before the engines halt) but drop the
    # all-engine barriers and the semaphore RANGE_CLEAR.  The NEFF epilogue
    # opens with its own all-engine barrier and then resets the entire
    # 256-semaphore space, so the tile-side clear+barriers only add ~1 us of
    # teardown inside the measured window.  Also split the drain's sem waits
    # onto nops (this walrus build allows one sync-wait command per
    # instruction).
    import concourse.tile as tile
    from concourse import mybir
    from concourse.vector_clock import ScopedClock

    if getattr(tile.TileContext._drain_and_barrier, "_is_patched", False):
        return

    def _patched_drain_and_barrier(self, tick_clock, wait_clock):
        nc = self.nc
        drain_inst = nc.sync.drain()
        wait_clock.add_sem_waits(
            drain_inst.ins, ScopedClock({None: tick_clock.global_clock})
        )
        ow = drain_inst.ins.sync_info.on_wait if drain_inst.ins.sync_info else None
        maxw = 1
        if ow and len(ow) > maxw:
            extra = list(ow[maxw:])
            del ow[maxw:]
            for i in range(0, len(extra), maxw):
                nop = nc.sync.nop(hint="drain_split", nofuse=True)
                if nop.ins.sync_info is None:
                    nop.ins.sync_info = mybir.SyncInfo(on_wait=[], on_update=[])
                for w in extra[i : i + maxw]:
                    nop.ins.sync_info.on_wait.append(w)
        assert self.sems is not None
        popped = nc._tile_sem_poison_stack.pop()
        assert popped is self._sem_poison
        # Python-side bookkeeping of clear_and_free_semaphores, with no
        # emitted instructions (the NEFF epilogue resets every semaphore).
        sems = list(self.sems.allocated().values())
        sem_nums = [s.num if hasattr(s, "num") else s for s in sems]
        nc._state.prepend_free_semaphores(sem_nums)
        for poison_set in nc._tile_sem_poison_stack:
            poison_set.update(sem_nums)

    _patched_drain_and_barrier._is_patched = True
    tile.TileContext._drain_and_barrier = _patched_drain_and_barrier


def _split_multi_waits(nc):
    """This walrus build allows one sync-wait command per instruction.

    Tile's sem assignment can attach several; move the extras onto nofuse
    NoOps inserted just before the instruction on the same engine (engines
    execute a block's instructions in order, so semantics are unchanged).
    """
    import bass_rust
    from concourse import mybir

    ctr = 0
    for f in nc.m.functions:
        for bb in f.blocks:
            new = []
            changed = False
            for inst in bb.instructions:
                si = inst.sync_info
                ow = si.on_wait if si else None
                if ow is not None and len(ow) > 1:
                    extra = list(ow[:-1])
                    del ow[:-1]
                    for w in extra:
                        ctr += 1
                        nop = bass_rust.InstNoOp()
                        nop.name = f"I-wsplit-{ctr}"
                        nop.engine = inst.engine
                        nop.sync_info = mybir.SyncInfo(on_wait=[w], on_update=[])
                        nop.bass_nofuse = True
                        new.append(nop)
                    changed = True
                new.append(inst)
            if changed:
                bb.instructions = new


def _chunk_sizes(cap):
    """Split cap token columns into chunks of <=512 (PSUM bank limit).

    The first chunk is as large as possible: it runs while the weights are
    still streaming in from HBM, and a wider chunk does more PE work per
    weight byte (lower demand bandwidth during the ramp)."""
    if cap <= 512:
        return [cap]
    first = 512
    rest = cap - first
    n = max(1, math.ceil(rest / 512))
    base = rest // n
    rem = rest - base * n
    return [first] + [base + (1 if i < rem else 0) for i in range(n)]


_NC_CACHE = {}


def _build_nc(cap):
    if cap in _NC_CACHE:
        return _NC_CACHE[cap]
    import concourse.bass as bass
    import concourse.tile as tile
    from concourse import mybir

    f32 = mybir.dt.float32
    bf16 = mybir.dt.bfloat16
    KD = D // 128  # 8  k-tiles over the model dim
    KH = H // 128  # 16 k-tiles over the hidden dim
    chunks = _chunk_sizes(cap)
    cmax = max(chunks)

    nc = bass.Bass()
    # 3D layouts so one dma_start can span several 128-row k-tiles.
    xT = nc.dram_tensor("xT", [KD, 128, cap], bf16, kind="ExternalInput")
    wgT = nc.dram_tensor("wgT", [KD, 128, H], bf16, kind="ExternalInput")
    wuT = nc.dram_tensor("wuT", [KD, 128, H], bf16, kind="ExternalInput")
    wdT = nc.dram_tensor("wdT", [KH, 128, D], bf16, kind="ExternalInput")
    out = nc.dram_tensor("out", [D, cap], bf16, kind="ExternalOutput")

    c_offs = []
    c0 = 0
    for cn in chunks:
        c_offs.append((c0, cn))
        c0 += cn

    GCOL = 768  # gate/up group-A column frontier (hi 0..5 of each k-tile)

    with tile.TileContext(nc) as tc:
        with (
            tc.tile_pool(name="wpool", bufs=1) as wpool,
            tc.tile_pool(name="hpool", bufs=2) as hpool,
            tc.tile_pool(name="opool", bufs=4) as opool,
            tc.tile_pool(name="psum", bufs=2, space="PSUM") as psum,
        ):
            x_all = wpool.tile([128, KD * cap], bf16, tag="x", name="x_all")
            wg_all = wpool.tile([128, KD * H], bf16, tag="wg", name="wg_all")
            wu_all = wpool.tile([128, KD * H], bf16, tag="wu", name="wu_all")
            wd_all = wpool.tile([128, KH * D], bf16, tag="wd", name="wd_all")
            zw = wpool.tile([128, 256], bf16, tag="zw", name="zw_sb")
            zs = wpool.tile([128, 16], bf16, tag="zs", name="zs_sb")
            nc.vector.memset(zw[:], 0.0)
            nc.vector.memset(zs[:, :8], 0.0)

            def xs(ki):
                return x_all[:, ki * cap : (ki + 1) * cap]

            def wgs(ki):
                return wg_all[:, ki * H : (ki + 1) * H]

            def wus(ki):
                return wu_all[:, ki * H : (ki + 1) * H]

            def wds(hk):
                return wd_all[:, hk * D : (hk + 1) * D]

            def dma_x(eng, a, b):
                dst = x_all[:, a * cap : b * cap]
                if b - a > 1:
                    dst = dst.rearrange("p (k c) -> p k c", k=b - a)
                    src = xT[a:b, :, :].transpose([1, 0, 2])
                else:
                    src = xT[a, :, :]
                eng.dma_start(dst, src)

            def dma_w(eng, w_all, wsrc, a, b, c0, c1, W=H):
                dst = w_all[:, a * W : b * W]
                if b - a > 1:
                    dst = dst.rearrange("p (k c) -> p k c", k=b - a)[:, :, c0:c1]
                    src = wsrc[a:b, :, c0:c1].transpose([1, 0, 2])
                else:
                    dst = dst[:, c0:c1]
                    src = wsrc[a, :, c0:c1]
                eng.dma_start(dst, src)

            # PE warmup on the "po" PSUM banks, whose first real use (the
            # down phase) is far away: the first real matmul takes no
            # dependency on these, so they purely absorb the 0.65/1.2 GHz
            # DVFS ramp during the DMA-wait head.
            warm = psum.tile([128, 512], f32, tag="po", name="warm")
            for _ in range(4):
                nc.tensor.matmul(
                    warm[:, :256], zw[:, :128], zw[:], start=True, stop=True
                )

            # --- input DMA plan, ordered by consumption deadline ---
            # SP: x, full-width rows (k-tile singles first for latency).
            dma_x(nc.sync, 0, 1)
            dma_x(nc.sync, 1, 2)
            dma_x(nc.sync, 2, 4)
            dma_x(nc.sync, 4, 6)
            dma_x(nc.sync, 6, 8)
            # ACT: gate weights for group A (cols 0:GCOL), k 0..3.
            dma_w(nc.scalar, wg_all, wgT, 0, 1, 0, GCOL)
            dma_w(nc.scalar, wg_all, wgT, 1, 2, 0, GCOL)
            dma_w(nc.scalar, wg_all, wgT, 2, 4, 0, GCOL)
            # GpSimd: gate weights group A, k 4..7.
            dma_w(nc.gpsimd, wg_all, wgT, 4, 8, 0, GCOL)
            # ACT loads its activation table lazily on the first ACTIVATE
            # (~1.3 us); trigger it on dummy data now so group 1's silus —
            # whose PSUM-bank releases gate group 2's matmuls — start sooner.
            nc.scalar.activation(
                zs[:, 8:16], zs[:, :8], mybir.ActivationFunctionType.Silu
            )
            # Gate weights groups B+C (cols GCOL:H), k-ordered across ACT/GpSimd.
            dma_w(nc.scalar, wg_all, wgT, 0, 2, GCOL, H)
            dma_w(nc.scalar, wg_all, wgT, 2, 4, GCOL, H)
            dma_w(nc.gpsimd, wg_all, wgT, 4, 8, GCOL, H)
            # SP: up weights, full rows (needed once chunk 0's gate phase ends).
            dma_w(nc.sync, wu_all, wuT, 0, 4, 0, H)
            dma_w(nc.sync, wu_all, wuT, 4, 8, 0, H)
            # GpSimd: down weights (needed last).
            dma_w(nc.gpsimd, wd_all, wdT, 0, 8, 0, D, W=D)
            dma_w(nc.gpsimd, wd_all, wdT, 8, 16, 0, D, W=D)

            def gate_up(c0, cn):
                # Phase 1: all gate matmuls; silu lands bf16 directly in h.
                # Phase 2: all up matmuls; h *= pu in place on the DVE.
                # Phasing delays the first need for wu by a whole gate phase.
                # Within a phase, k is the OUTER loop over groups of 6 h-tiles
                # accumulating in 6 PSUM banks: weight consumption order then
                # matches the k-major DMA arrival order, so the PE never
                # outruns the transfer frontier during the startup ramp.
                h_sb = hpool.tile([128, KH * cmax], bf16, tag="h", name="h_sb")
                csl = slice(c0, c0 + cn)

                def phase(wsl, writer):
                    for g0 in range(0, KH, 6):
                        his = range(g0, min(g0 + 6, KH))
                        pp = [
                            psum.tile(
                                [128, 512], f32, tag=f"pp{j}", bufs=1, name=f"pp{j}"
                            )
                            for j in range(len(his))
                        ]
                        for ki in range(KD):
                            for j, hi in enumerate(his):
                                nc.tensor.matmul(
                                    pp[j][:, :cn],
                                    wsl(ki)[:, 128 * hi : 128 * (hi + 1)],
                                    xs(ki)[:, csl],
                                    start=(ki == 0),
                                    stop=(ki == KD - 1),
                                )
                        for j, hi in enumerate(his):
                            writer(hi, pp[j])

                def gate_writer(hi, pp):
                    nc.scalar.activation(
                        h_sb[:, cmax * hi : cmax * hi + cn],
                        pp[:, :cn],
                        mybir.ActivationFunctionType.Silu,
                    )

                def up_writer(hi, pp):
                    hslc = slice(cmax * hi, cmax * hi + cn)
                    nc.vector.tensor_mul(h_sb[:, hslc], h_sb[:, hslc], pp[:, :cn])

                phase(wgs, gate_writer)
                phase(wus, up_writer)
                return h_sb

            def down(h_sb, c0, cn):
                last_chunk = (c0, cn) == c_offs[-1]
                for di in range(KD):
                    dsl = slice(128 * di, 128 * (di + 1))
                    # The very last d-tile runs as two column groups so its
                    # cast+store pipelines against its own matmuls instead of
                    # sitting fully exposed after the final one.
                    if last_chunk and di == KD - 1:
                        cgroups = [(0, cn // 2), (cn // 2, cn)]
                    else:
                        cgroups = [(0, cn)]
                    for g0, g1 in cgroups:
                        gw = g1 - g0
                        po = psum.tile([128, 512], f32, tag="po", name="po")
                        for hk in range(KH):
                            nc.tensor.matmul(
                                po[:, :gw],
                                wds(hk)[:, dsl],
                                h_sb[:, cmax * hk + g0 : cmax * hk + g1],
                                start=(hk == 0),
                                stop=(hk == KH - 1),
                            )
                        o = opool.tile([128, 512], bf16, tag="o", name="o")
                        nc.vector.tensor_copy(o[:, :gw], po[:, :gw])
                        # Split across SP and ACT so the two issues (0.6 us
                        # of serial sequencer time each) overlap.
                        mid = gw // 2
                        nc.sync.dma_start(
                            out[dsl, c0 + g0 : c0 + g0 + mid], o[:, :mid]
                        )
                        nc.scalar.dma_start(
                            out[dsl, c0 + g0 + mid : c0 + g1], o[:, mid:gw]
                        )

            # Software-pipelined emission: down(c) goes after gate_up(c+1) so
            # the PE can run chunk c+1's gate matmuls while the DVE finishes
            # chunk c's h tiles (h is double-buffered).
            prev = None
            for c0i, cni in c_offs:
                h_sb = gate_up(c0i, cni)
                if prev is not None:
                    down(*prev)
                prev = (h_sb, c0i, cni)
            down(*prev)
    _split_multi_waits(nc)
    _NC_CACHE[cap] = nc
    return nc


def kernel(x, expert_indices, w_gate, w_up, w_down):
    global LAST_RESULT
    _install_shims()
    from concourse import bass_utils

    x = np.asarray(x)
    ei = np.asarray(expert_indices).astype(np.int64)
    w_gate = np.asarray(w_gate)
    w_up = np.asarray(w_up)
    w_down = np.asarray(w_down)

    flat = ei.reshape(-1)  # pair p = t*A + a  ->  expert id
    # Dedup: a (token, slot) pair whose expert already appears in an earlier
    # slot of the same token produces an identical output row — compute the
    # first occurrence only and copy the result to the duplicates afterward.
    keep = np.ones(T * A, dtype=bool)
    for a in range(1, A):
        dup_any = np.zeros(T, dtype=bool)
        for b in range(a):
            dup_any |= ei[:, a] == ei[:, b]
        keep[a::A] = ~dup_any[:T]
    kept = np.nonzero(keep)[0]
    flat_kept = flat[kept]
    counts = np.bincount(flat_kept, minlength=E)
    order = np.argsort(flat_kept, kind="stable")
    starts = np.zeros(E + 1, dtype=np.int64)
    np.cumsum(counts, out=starts[1:])
    cap = int(counts.max())
    cap = max(cap, 128)

    KD = D // 128
    KH = H // 128
    idx_per_core = []
    in_maps = []
    for e in range(E):
        idx = kept[order[starts[e] : starts[e + 1]]]  # original pair ids
        idx_per_core.append(idx)
        tok = idx // A
        xeT = np.zeros((D, cap), dtype=BF16)
        xeT[:, : len(idx)] = x[tok].T.astype(BF16)
        in_maps.append(
            {
                "xT": xeT.reshape(KD, 128, cap),
                "wgT": np.ascontiguousarray(w_gate[e].T)
                .astype(BF16)
                .reshape(KD, 128, H),
                "wuT": np.ascontiguousarray(w_up[e].T)
                .astype(BF16)
                .reshape(KD, 128, H),
                "wdT": np.ascontiguousarray(w_down[e].T)
                .astype(BF16)
                .reshape(KH, 128, D),
            }
        )

    nc = _build_nc(cap)
    res = bass_utils.run_bass_kernel_spmd(nc, in_maps, core_ids=list(range(N_CORES)))
    LAST_RESULT = res

    out = np.zeros((T * A, D), dtype=np.float32)
    for e in range(E):
        idx = idx_per_core[e]
        oT = np.asarray(res.results[e]["out"])  # [D, cap] bf16
        out[idx] = oT[:, : len(idx)].T.astype(np.float32)
    out = out.reshape(T, A, D)
    for a in range(1, A):  # fill duplicate slots from their first occurrence
        for b in range(a):
            m = ei[:, a] == ei[:, b]
            if b > 0:
                for c in range(b):
                    m &= ei[:, b] != ei[:, c]  # b is itself the first occurrence
            out[m, a] = out[m, b]
    return out
